# revision 33
# baseline (speedup 1.0000x reference)
"""Trainium2 Bass kernel for a 3-layer stacked GRU + dual masked-linear heads.

Model (PyTorch GRUCell semantics, eval mode):
    h1,h2,h3 : 3 chained GRUCell layers over T=512 steps (B=32, F_IN=513, H=512)
    s1 = relu(h3_seq @ W_l1.T + b_l1); s2 = relu(h3_seq @ W_l2.T + b_l2)
    m1 = s1/(s1+s2+1e-16); m2 = s2/(s1+s2+1e-16)
    returns (m1*x, m2*x)

v2 design (vs the original 160 ms/13.7 ms-device baseline):
  - L-layout per step: [p = 32q+b, f = 128g+j] (q = H quarter, b = batch,
    g = gate r/z/n, j = offset). Matmuls keep h^T stationary, stream f16
    weights through 4 concurrently-running PE column groups.
  - Both matmul sides of a cell accumulate into ONE PSUM pair (PA gets
    gi+bias on 384 cols plus the recurrent r|z on 0:256; PB gets the
    recurrent n part), so the sigmoid reads PSUM directly and the old
    rz-add + staging copy disappear.  Biases enter as K=1 ones-row matmul
    rounds (layer 1's ride the padded x k-tile); only b_hh*n needs a
    per-step DVE add (it sits inside the r* product).
  - x is streamed in 4-step blocks; h transposes collect in 4-step stack
    tiles (which also serve as matmul stationaries); h3 stacks go to DRAM
    once per block.  No per-step DMA anywhere.
  - h3 transposes stream to DRAM per block; the output phase consumes them
    as (t,b)-partition GEMMs with the mask math split across DVE/ACT.
"""

import os
import numpy as np

B, T, F, H = 32, 512, 513, 512
BLK = 4
NCORES = 8

_CACHE = {}


# ---------------------------------------------------------------------------
# Host-side repacking
# ---------------------------------------------------------------------------

def _moving(W):
    """W [3H, K] (K mult of 128) -> [128, KT, 4, 384] f16 moving tiles.

    out[kk, kt, q, 128*g+j] = W[g*512 + 128*q + j, 128*kt + kk]
    """
    K = W.shape[1]
    KT = K // 128
    Wk = W.reshape(3, 4, 128, KT, 128)  # [g, q, j, kt, kk]
    return np.ascontiguousarray(
        np.transpose(Wk, (4, 3, 1, 0, 2)).reshape(128, KT, 4, 384).astype(np.float16))


def _gate_rows(v):
    """v [3H] -> [4, 384] rows in (q, 128g+j) order."""
    return np.ascontiguousarray(
        np.transpose(v.reshape(3, 4, 128), (1, 0, 2)).reshape(4, 384).astype(np.float32))


def prep_inputs(inputs, t_steps):
    x = np.asarray(inputs["x"], np.float32)
    t_total = t_steps
    nblk = t_total // BLK
    p = {}

    f32 = np.float32

    # recurrent weights [128, 3, 4, 4, 384]
    p["Wh"] = np.ascontiguousarray(np.stack(
        [_moving(np.asarray(inputs[f"W_hh{l}"], f32)) for l in (1, 2, 3)], axis=1))
    # input-side weights for layers 2,3  [128, 2, 4, 4, 384]
    p["Wi"] = np.ascontiguousarray(np.stack(
        [_moving(np.asarray(inputs[f"W_ih{l}"], f32)) for l in (2, 3)], axis=1))

    # layer-1 x weights [128, 5, 4, 384]; kt=4 packs [W[:,512]; bias_l1]
    W1 = np.asarray(inputs["W_ih1"], f32)
    bi1 = np.asarray(inputs["b_ih1"], f32)
    bh1 = np.asarray(inputs["b_hh1"], f32)
    bias_l1 = bi1.copy()
    bias_l1[:2 * H] += bh1[:2 * H]          # fold b_hh r,z into the gi bias
    Wx = np.zeros((128, 5, 4, 384), np.float16)
    Wx[:, :4] = _moving(W1[:, :512])
    Wx[0, 4] = _gate_rows(W1[:, 512])
    Wx[1, 4] = _gate_rows(bias_l1)
    p["Wx"] = Wx

    # gi bias rows for layers 2,3: [1, 2, 4, 384] f16
    b23 = np.zeros((1, 2, 4, 384), np.float16)
    for li, l in enumerate((2, 3)):
        bi = np.asarray(inputs[f"b_ih{l}"], f32)
        bh = np.asarray(inputs[f"b_hh{l}"], f32)
        bb = bi.copy()
        bb[:2 * H] += bh[:2 * H]
        b23[0, li] = _gate_rows(bb).astype(np.float16)
    p["b23"] = b23

    # b_hh*n broadcast tiles [128, 3, 128] f32: bc[32q+b, l, j] = b_hh[2H+128q+j]
    bc = np.zeros((128, 3, 128), f32)
    for li, l in enumerate((1, 2, 3)):
        bn = np.asarray(inputs[f"b_hh{l}"], f32)[2 * H:].reshape(4, 128)
        bc[:, li, :] = np.repeat(bn, 32, axis=0)
    p["bc"] = bc

    # x blocks for gi1: [nblk, 128, 5, 128] f16
    xb = np.zeros((nblk, 128, 5, 128), np.float16)
    # [blk, kk, kt, 32t'+b] = x[b, 4blk+t', 128kt+kk]
    xr = x[:, :t_total, :512].reshape(B, nblk, BLK, 4, 128)
    xb[:, :, :4, :] = np.transpose(xr, (1, 4, 3, 2, 0)).reshape(nblk, 128, 4, BLK * 32)
    xb[:, 0, 4, :] = x[:, :t_total, 512].reshape(B, nblk, BLK).transpose(1, 2, 0).reshape(nblk, 128)
    xb[:, 1, 4, :] = 1.0
    p["xb"] = np.ascontiguousarray(xb)

    # output head weights [128, 2, 4, 640] f16 (f padded to 640)
    Wl = np.zeros((128, 2, 4, 640), np.float16)
    for i in (1, 2):
        Wl[:, i - 1, :, :513] = np.transpose(
            np.asarray(inputs[f"W_l{i}"], f32).reshape(513, 4, 128), (2, 1, 0))
    p["Wl"] = np.ascontiguousarray(Wl)
    bl = np.zeros((1, 2, 640), np.float16)
    for i in (1, 2):
        bl[0, i - 1, :513] = np.asarray(inputs[f"b_l{i}"], f32)
    p["bl"] = bl

    # x for masking, (t,b)-partition layout [nblk, 128, 640] f32
    xo = np.zeros((nblk, 128, 640), f32)
    xo[:, :, :513] = np.transpose(
        x[:, :t_total, :].reshape(B, nblk, BLK, 513), (1, 2, 0, 3)).reshape(nblk, 128, 513)
    p["xo"] = np.ascontiguousarray(xo)
    return p


# ---------------------------------------------------------------------------
# Device kernel
# ---------------------------------------------------------------------------

def build_nc(t_steps):
    from contextlib import ExitStack
    import concourse.bacc as bacc
    import concourse.mybir as mybir
    import concourse.tile as tile
    from concourse.masks import make_identity

    f32 = mybir.dt.float32
    f16 = mybir.dt.float16
    AF = mybir.ActivationFunctionType
    ALU = mybir.AluOpType

    t_total = t_steps
    nblk = t_total // BLK
    nc = bacc.Bacc("TRN2", target_bir_lowering=False)

    # ---- DRAM I/O -------------------------------------------------------
    Wh_d = nc.dram_tensor("Wh", [128, 3, 4, 4, 384], f16, kind="ExternalInput")
    Wi_d = nc.dram_tensor("Wi", [128, 2, 4, 4, 384], f16, kind="ExternalInput")
    Wx_d = nc.dram_tensor("Wx", [128, 5, 4, 384], f16, kind="ExternalInput")
    b23_d = nc.dram_tensor("b23", [1, 2, 4, 384], f16, kind="ExternalInput")
    bc_d = nc.dram_tensor("bc", [128, 3, 128], f32, kind="ExternalInput")
    xb_d = nc.dram_tensor("xb", [nblk, 128, 5, 128], f16, kind="ExternalInput")
    Wl_d = nc.dram_tensor("Wl", [128, 2, 4, 640], f16, kind="ExternalInput")
    bl_d = nc.dram_tensor("bl", [1, 2, 640], f16, kind="ExternalInput")
    xo_d = nc.dram_tensor("xo", [nblk, 128, 640], f32, kind="ExternalInput")
    o1_d = nc.dram_tensor("o1", [nblk, 128, 640], f32, kind="ExternalOutput")
    o2_d = nc.dram_tensor("o2", [nblk, 128, 640], f32, kind="ExternalOutput")

    with ExitStack() as ctx:
        tc = ctx.enter_context(tile.TileContext(nc))

        consts = ctx.enter_context(tc.tile_pool(name="consts", bufs=1))
        ident = consts.tile([128, 128], f32)
        make_identity(nc, ident)
        ones = consts.tile([1, 128], f16)
        nc.vector.memset(ones, 1.0)
        zeros = consts.tile([128, 128], f32)
        nc.vector.memset(zeros, 0.0)

        dram = ctx.enter_context(tc.tile_pool(name="dram", bufs=1, space="DRAM"))
        h3T = dram.tile([nblk, 128, BLK, 128], f16)

        with ExitStack() as rctx:
            wrec = rctx.enter_context(tc.tile_pool(name="wrec", bufs=1))
            Wh = wrec.tile([128, 3, 4, 4, 384], f16)
            nc.sync.dma_start(out=Wh, in_=Wh_d[:, :, :, :, :])
            Wi = wrec.tile([128, 2, 4, 4, 384], f16)
            nc.sync.dma_start(out=Wi, in_=Wi_d[:, :, :, :, :])
            Wx = wrec.tile([128, 5, 4, 384], f16)
            nc.sync.dma_start(out=Wx, in_=Wx_d[:, :, :, :])
            b23 = wrec.tile([1, 2, 4, 384], f16)
            nc.sync.dma_start(out=b23, in_=b23_d[:, :, :, :])
            bc = wrec.tile([128, 3, 128], f32)
            nc.sync.dma_start(out=bc, in_=bc_d[:, :, :])

            xpool = rctx.enter_context(tc.tile_pool(name="xpool", bufs=3))
            hTs_p = rctx.enter_context(tc.tile_pool(name="hTs", bufs=3))
            hLp = rctx.enter_context(tc.tile_pool(name="hLp", bufs=2))
            gpool = rctx.enter_context(tc.tile_pool(name="gpool", bufs=2))
            pap = rctx.enter_context(tc.tile_pool(name="pap", bufs=1, space="PSUM"))
            pbp = rctx.enter_context(tc.tile_pool(name="pbp", bufs=1, space="PSUM"))
            tpp = rctx.enter_context(tc.tile_pool(name="tpp", bufs=2, space="PSUM"))

            # per-layer state: transposed-h stacks keyed by (layer, block)
            hT_stacks = {}
            hL_cur = {1: None, 2: None, 3: None}   # [128,128] f32 last h
            xb_cur = {}

            def emit_step(l, blk, tp_idx, first, xbt):
                """One GRU step for layer l; tp_idx = t % BLK.

                PA [128,384] accumulates gi (+bias) on all 384 cols and the
                recurrent r|z part on cols 0:256.  PB [128,128] holds the
                recurrent n part (kept separate for the r* product).
                """
                li = l - 1
                hL_prev = hL_cur[l] if not first else zeros

                PA = pap.tile([128, 384], f32, tag=f"pa{l}")

                # ---- input side ----
                if l == 1:
                    for kt in range(5):
                        kk = 128 if kt < 4 else 2
                        for q in range(4):
                            nc.tensor.matmul(
                                PA[32 * q:32 * q + 32, :],
                                xbt[:kk, kt, 32 * tp_idx:32 * tp_idx + 32],
                                Wx[:kk, kt, q, :],
                                start=(kt == 0), stop=(first and kt == 4),
                                tile_position=(0, 32 * q),
                                skip_group_check=True)
                else:
                    hT_in = hT_stacks[(l - 1, blk)]
                    for q in range(4):
                        nc.tensor.matmul(
                            PA[32 * q:32 * q + 32, :],
                            ones[0:1, 32 * q:32 * q + 32],
                            b23[0:1, l - 2, q, :],
                            start=True, stop=False,
                            tile_position=(0, 32 * q),
                            skip_group_check=True)
                    for kt in range(4):
                        for q in range(4):
                            nc.tensor.matmul(
                                PA[32 * q:32 * q + 32, :],
                                hT_in[:, tp_idx, 32 * kt:32 * kt + 32],
                                Wi[:, l - 2, kt, q, :],
                                start=False, stop=(first and kt == 3),
                                tile_position=(0, 32 * q),
                                skip_group_check=True)

                # ---- recurrent side ----
                if not first:
                    PB = pbp.tile([128, 128], f32, tag=f"pb{l}")
                    if tp_idx > 0:
                        hT_own = hT_stacks[(l, blk)][:, tp_idx - 1, :]
                    else:
                        hT_own = hT_stacks[(l, blk - 1)][:, BLK - 1, :]
                    for kt in range(4):
                        for q in range(4):
                            nc.tensor.matmul(
                                PA[32 * q:32 * q + 32, 0:256],
                                hT_own[:, 32 * kt:32 * kt + 32],
                                Wh[:, li, kt, q, 0:256],
                                start=False, stop=(kt == 3),
                                tile_position=(0, 32 * q),
                                skip_group_check=True)
                            nc.tensor.matmul(
                                PB[32 * q:32 * q + 32, :],
                                hT_own[:, 32 * kt:32 * kt + 32],
                                Wh[:, li, kt, q, 256:384],
                                start=(kt == 0), stop=(kt == 3),
                                tile_position=(0, 32 * q))

                # ---- gates ----
                rz = gpool.tile([128, 256], f32, tag=f"rz{l}")
                nc.scalar.activation(rz, PA[:, 0:256], AF.Sigmoid)

                rn = gpool.tile([128, 128], f32, tag=f"rn{l}")
                if first:
                    nc.vector.tensor_mul(rn, rz[:, 0:128], bc[:, li, :])
                else:
                    p1n = gpool.tile([128, 128], f32, tag=f"p1n{l}")
                    nc.vector.tensor_add(p1n, PB, bc[:, li, :])
                    nc.vector.tensor_mul(rn, rz[:, 0:128], p1n)
                n = gpool.tile([128, 128], f32, tag=f"n{l}")
                nc.vector.tensor_add(n, rn, PA[:, 256:384])
                nc.scalar.activation(n, n, AF.Tanh)

                d = gpool.tile([128, 128], f32, tag=f"d{l}")
                nc.vector.tensor_sub(d, hL_prev, n)
                zd = gpool.tile([128, 128], f32, tag=f"zd{l}")
                nc.vector.tensor_mul(zd, rz[:, 128:256], d)
                hL = hLp.tile([128, 128], f32, tag=f"hL{l}")
                nc.vector.tensor_add(hL, n, zd)
                hL_cur[l] = hL

                tp = tpp.tile([128, 128], f32, tag="tp")
                nc.tensor.transpose(tp, hL, ident)
                stack = hT_stacks[(l, blk)]
                if l == 2:
                    nc.vector.tensor_copy(stack[:, tp_idx, :], tp)
                else:
                    nc.scalar.copy(stack[:, tp_idx, :], tp)

            # ---- wavefront over supersteps ----
            def load_xb(blk):
                xbt = xpool.tile([128, 5, 128], f16, tag="xb")
                nc.sync.dma_start(out=xbt, in_=xb_d[blk, :, :, :])
                xb_cur[blk] = xbt
                xb_cur.pop(blk - 3, None)

            load_xb(0)
            for s in range(nblk + 2):
                if s + 1 < nblk:
                    load_xb(s + 1)
                active = [l for l in (1, 2, 3) if 0 <= s - (l - 1) < nblk]
                for l in active:
                    blk = s - (l - 1)
                    hT_stacks[(l, blk)] = hTs_p.tile(
                        [128, BLK, 128], f16, tag=f"hTs{l}", name=f"hTs{l}")
                    hT_stacks.pop((l, blk - 3), None)
                # interleave cells across layers so each layer's serial
                # gate-chain latency is covered by the other layers' matmuls
                for tp_idx in range(BLK):
                    for l in active:
                        blk = s - (l - 1)
                        t = BLK * blk + tp_idx
                        emit_step(l, blk, tp_idx, first=(t == 0),
                                  xbt=xb_cur.get(blk))
                if 3 in active:
                    blk = s - 2
                    nc.sync.dma_start(out=h3T[blk, :, :, :], in_=hT_stacks[(3, blk)])

        # ---- output phase ----------------------------------------------
        with ExitStack() as octx:
            wout = octx.enter_context(tc.tile_pool(name="wout", bufs=1))
            Wl = wout.tile([128, 2, 4, 640], f16)
            nc.sync.dma_start(out=Wl, in_=Wl_d[:, :, :, :])
            bl = wout.tile([1, 2, 640], f16)
            nc.sync.dma_start(out=bl, in_=bl_d[:, :, :])

            hpo = octx.enter_context(tc.tile_pool(name="hpo", bufs=3))
            xop = octx.enter_context(tc.tile_pool(name="xop", bufs=2))
            sp = octx.enter_context(tc.tile_pool(name="sp", bufs=2))
            osum = octx.enter_context(tc.tile_pool(name="osum", bufs=2, space="PSUM"))

            for blk in range(nblk):
                # relayout so chunk kt is contiguous: hT3[k, kt, t', b]
                hT3 = hpo.tile([128, 4, BLK, 32], f16, tag="hT3")
                for kt in range(4):
                    nc.sync.dma_start(
                        out=hT3[:, kt, :, :],
                        in_=h3T[blk, :, :, 32 * kt:32 * kt + 32])
                xo = xop.tile([128, 640], f32, tag="xo")
                nc.sync.dma_start(out=xo, in_=xo_d[blk, :, :])

                ss = []
                for hd in range(2):
                    psa = osum.tile([128, 512], f32, tag=f"psa{hd}")
                    psb = osum.tile([128, 128], f32, tag=f"psb{hd}")
                    nc.tensor.matmul(psa, ones[0:1, :], bl[0:1, hd, 0:512],
                                     start=True, stop=False)
                    nc.tensor.matmul(psb, ones[0:1, :], bl[0:1, hd, 512:640],
                                     start=True, stop=False)
                    for kt in range(4):
                        hs = hT3[:, kt, :, :].rearrange("k t b -> k (t b)")
                        nc.tensor.matmul(psa, hs, Wl[:, hd, kt, 0:512],
                                         start=False, stop=(kt == 3))
                        nc.tensor.matmul(psb, hs, Wl[:, hd, kt, 512:640],
                                         start=False, stop=(kt == 3))
                    s = sp.tile([128, 640], f32, tag=f"s{hd}")
                    nc.vector.tensor_scalar_max(s[:, 0:512], psa, 0.0)
                    nc.vector.tensor_scalar_max(s[:, 512:640], psb, 0.0)
                    ss.append(s)

                den = sp.tile([128, 640], f32, tag="den")
                nc.vector.tensor_add(den, ss[0], ss[1])
                nc.vector.tensor_scalar_add(den, den, 1e-16)
                lden = sp.tile([128, 640], f32, tag="lden")
                nc.scalar.activation(lden, den, AF.Ln)
                rden = sp.tile([128, 640], f32, tag="rden")
                nc.scalar.activation(rden, lden, AF.Exp, scale=-1.0)
                xr = sp.tile([128, 640], f32, tag="xr")
                nc.vector.tensor_mul(xr, xo, rden)
                for hd, od in ((0, o1_d), (1, o2_d)):
                    o = sp.tile([128, 640], f32, tag=f"o{hd}")
                    nc.vector.tensor_mul(o, ss[hd], xr)
                    nc.sync.dma_start(out=od[blk, :, :], in_=o)

    nc.finalize()
    return nc


# ---------------------------------------------------------------------------
# Runner (compiled-executable cache, SPMD via shard_map)
# ---------------------------------------------------------------------------

class _Runner:
    def __init__(self, nc, n_cores):
        import jax
        import concourse.mybir as mybir
        from concourse import bass2jax
        from concourse.bass2jax import (
            _bass_exec_p, install_neuronx_cc_hook, partition_id_tensor)
        from jax.experimental.shard_map import shard_map
        from jax.sharding import Mesh, PartitionSpec

        install_neuronx_cc_hook()
        self.jax = jax
        self.nc = nc
        self.n_cores = n_cores
        partition_name = (nc.partition_id_tensor.name
                          if nc.partition_id_tensor else None)
        in_names, out_names, out_avals, zero_outs = [], [], [], []
        for alloc in nc.m.functions[0].allocations:
            if not isinstance(alloc, mybir.MemoryLocationSet):
                continue
            name = alloc.memorylocations[0].name
            if alloc.kind == "ExternalInput":
                if name != partition_name:
                    in_names.append(name)
            elif alloc.kind == "ExternalOutput":
                shape = tuple(alloc.tensor_shape)
                dtype = mybir.dt.np(alloc.dtype)
                out_names.append(name)
                out_avals.append(jax.core.ShapedArray(shape, dtype))
                zero_outs.append(np.zeros(shape, dtype))
        n_params = len(in_names)
        self.in_names = list(in_names)
        self.out_names = out_names
        self.out_avals = out_avals
        self.zero_outs = zero_outs
        all_in = in_names + out_names
        if partition_name is not None:
            all_in.append(partition_name)

        def _body(*args):
            operands = list(args)
            if partition_name is not None:
                operands.append(partition_id_tensor())
            return tuple(_bass_exec_p.bind(
                *operands, out_avals=tuple(out_avals), in_names=tuple(all_in),
                out_names=tuple(out_names), lowering_input_output_aliases=(),
                sim_require_finite=True, sim_require_nnan=True, nc=nc))

        devices = jax.devices()[:n_cores]
        self.mesh = Mesh(np.asarray(devices), ("core",))
        self.pspec = PartitionSpec("core")
        n_out = len(out_names)
        self.sharded = jax.jit(
            shard_map(_body, mesh=self.mesh,
                      in_specs=(self.pspec,) * (n_params + n_out),
                      out_specs=(self.pspec,) * n_out,
                      check_rep=False),
            keep_unused=True)

    def prepare(self, in_map):
        import jax
        from jax.sharding import NamedSharding
        sh = NamedSharding(self.mesh, self.pspec)
        args = [np.concatenate([np.asarray(in_map[n])] * self.n_cores, axis=0)
                for n in self.in_names]
        args += [np.zeros((self.n_cores * z.shape[0], *z.shape[1:]), z.dtype)
                 for z in self.zero_outs]
        return [jax.device_put(a, sh) for a in args]

    def call(self, concat_in):
        return self.sharded(*concat_in)

    def results0(self, outs):
        """Core-0 slice of each output (sliced on device before transfer)."""
        res = {}
        for i, name in enumerate(self.out_names):
            n0 = self.out_avals[i].shape[0]
            res[name] = np.asarray(outs[i][:n0])
        return res


def _get_runner(t_steps):
    if t_steps not in _CACHE:
        _CACHE[t_steps] = _Runner(build_nc(t_steps), NCORES)
    return _CACHE[t_steps]


def _run(inputs, t_steps=T, time_reps=0):
    import time as _time
    r = _get_runner(t_steps)
    p = prep_inputs(inputs, t_steps)
    concat_in = r.prepare(p)
    outs = r.call(concat_in)  # first call compiles
    out = r.results0(outs)
    o1 = _unpack_out(out["o1"], t_steps)
    o2 = _unpack_out(out["o2"], t_steps)

    times = []
    for _ in range(time_reps):
        t0 = _time.time()
        outs = r.call(concat_in)
        for o in outs:
            o.block_until_ready()
        times.append(_time.time() - t0)
    return (o1, o2), times


def _unpack_out(o, t_steps):
    """[nblk, 32t'+b, 640] -> [B, T, 513]."""
    nblk = t_steps // BLK
    o = o.reshape(nblk, BLK, B, 640)[:, :, :, :F]
    return np.ascontiguousarray(np.transpose(o, (2, 0, 1, 3)).reshape(B, t_steps, F))


def kernel(**inputs):
    (o1, o2), _ = _run(inputs, T)
    return (o1, o2)


# revision 35
# speedup vs baseline: 1.6129x; 1.6129x over previous
"""Trainium2 Bass kernel for a 3-layer stacked GRU + dual masked-linear heads.

Model (PyTorch GRUCell semantics, eval mode):
    h1,h2,h3 : 3 chained GRUCell layers over T=512 steps (B=32, F_IN=513, H=512)
    s1 = relu(h3_seq @ W_l1.T + b_l1); s2 = relu(h3_seq @ W_l2.T + b_l2)
    m1 = s1/(s1+s2+1e-16); m2 = s2/(s1+s2+1e-16)
    returns (m1*x, m2*x)

v2 design (vs the original 160 ms/13.7 ms-device baseline):
  - L-layout per step: [p = 32q+b, f = 128g+j] (q = H quarter, b = batch,
    g = gate r/z/n, j = offset). Matmuls keep h^T stationary, stream f16
    weights through 4 concurrently-running PE column groups.
  - Both matmul sides of a cell accumulate into ONE PSUM pair (PA gets
    gi+bias on 384 cols plus the recurrent r|z on 0:256; PB gets the
    recurrent n part), so the sigmoid reads PSUM directly and the old
    rz-add + staging copy disappear.  Biases enter as K=1 ones-row matmul
    rounds (layer 1's ride the padded x k-tile); only b_hh*n needs a
    per-step DVE add (it sits inside the r* product).
  - x is streamed in 4-step blocks; h transposes collect in 4-step stack
    tiles (which also serve as matmul stationaries); h3 stacks go to DRAM
    once per block.  No per-step DMA anywhere.
  - h3 transposes stream to DRAM per block; the output phase consumes them
    as (t,b)-partition GEMMs with the mask math split across DVE/ACT.
"""

import os
import numpy as np

B, T, F, H = 32, 512, 513, 512
BLK = 4
NCORES = 8

_CACHE = {}


# ---------------------------------------------------------------------------
# Host-side repacking
# ---------------------------------------------------------------------------

def _moving(W):
    """W [3H, K] (K mult of 128) -> [128, KT, 4, 384] f16 moving tiles.

    out[kk, kt, q, 128*g+j] = W[g*512 + 128*q + j, 128*kt + kk]
    """
    K = W.shape[1]
    KT = K // 128
    Wk = W.reshape(3, 4, 128, KT, 128)  # [g, q, j, kt, kk]
    return np.ascontiguousarray(
        np.transpose(Wk, (4, 3, 1, 0, 2)).reshape(128, KT, 4, 384).astype(np.float16))


def _gate_rows(v):
    """v [3H] -> [4, 384] rows in (q, 128g+j) order."""
    return np.ascontiguousarray(
        np.transpose(v.reshape(3, 4, 128), (1, 0, 2)).reshape(4, 384).astype(np.float32))


def prep_inputs(inputs, t_steps):
    x = np.asarray(inputs["x"], np.float32)
    t_total = t_steps
    nblk = t_total // BLK
    p = {}

    f32 = np.float32

    # recurrent weights [128, 3, 4, 4, 384]
    p["Wh"] = np.ascontiguousarray(np.stack(
        [_moving(np.asarray(inputs[f"W_hh{l}"], f32)) for l in (1, 2, 3)], axis=1))
    # input-side weights for layers 2,3  [128, 2, 4, 4, 384]
    p["Wi"] = np.ascontiguousarray(np.stack(
        [_moving(np.asarray(inputs[f"W_ih{l}"], f32)) for l in (2, 3)], axis=1))

    # layer-1 x weights [128, 5, 4, 384]; kt=4 packs [W[:,512]; bias_l1]
    W1 = np.asarray(inputs["W_ih1"], f32)
    bi1 = np.asarray(inputs["b_ih1"], f32)
    bh1 = np.asarray(inputs["b_hh1"], f32)
    bias_l1 = bi1.copy()
    bias_l1[:2 * H] += bh1[:2 * H]          # fold b_hh r,z into the gi bias
    Wx = np.zeros((128, 5, 4, 384), np.float16)
    Wx[:, :4] = _moving(W1[:, :512])
    Wx[0, 4] = _gate_rows(W1[:, 512])
    Wx[1, 4] = _gate_rows(bias_l1)
    p["Wx"] = Wx

    # gi bias rows for layers 2,3: [1, 2, 4, 384] f16
    b23 = np.zeros((1, 2, 4, 384), np.float16)
    for li, l in enumerate((2, 3)):
        bi = np.asarray(inputs[f"b_ih{l}"], f32)
        bh = np.asarray(inputs[f"b_hh{l}"], f32)
        bb = bi.copy()
        bb[:2 * H] += bh[:2 * H]
        b23[0, li] = _gate_rows(bb).astype(np.float16)
    p["b23"] = b23

    # b_hh*n broadcast tiles [128, 3, 128] f32: bc[32q+b, l, j] = b_hh[2H+128q+j]
    bc = np.zeros((128, 3, 128), f32)
    for li, l in enumerate((1, 2, 3)):
        bn = np.asarray(inputs[f"b_hh{l}"], f32)[2 * H:].reshape(4, 128)
        bc[:, li, :] = np.repeat(bn, 32, axis=0)
    p["bc"] = bc

    # x blocks for gi1: [nblk, 128, 5, 128] f16
    xb = np.zeros((nblk, 128, 5, 128), np.float16)
    # [blk, kk, kt, 32t'+b] = x[b, 4blk+t', 128kt+kk]
    xr = x[:, :t_total, :512].reshape(B, nblk, BLK, 4, 128)
    xb[:, :, :4, :] = np.transpose(xr, (1, 4, 3, 2, 0)).reshape(nblk, 128, 4, BLK * 32)
    xb[:, 0, 4, :] = x[:, :t_total, 512].reshape(B, nblk, BLK).transpose(1, 2, 0).reshape(nblk, 128)
    xb[:, 1, 4, :] = 1.0
    p["xb"] = np.ascontiguousarray(xb)

    # output head weights [128, 2, 4, 640] f16 (f padded to 640)
    Wl = np.zeros((128, 2, 4, 640), np.float16)
    for i in (1, 2):
        Wl[:, i - 1, :, :513] = np.transpose(
            np.asarray(inputs[f"W_l{i}"], f32).reshape(513, 4, 128), (2, 1, 0))
    p["Wl"] = np.ascontiguousarray(Wl)
    bl = np.zeros((1, 2, 640), np.float16)
    for i in (1, 2):
        bl[0, i - 1, :513] = np.asarray(inputs[f"b_l{i}"], f32)
    p["bl"] = bl

    # x for masking, (t,b)-partition layout [nblk, 128, 640] f32
    xo = np.zeros((nblk, 128, 640), f32)
    xo[:, :, :513] = np.transpose(
        x[:, :t_total, :].reshape(B, nblk, BLK, 513), (1, 2, 0, 3)).reshape(nblk, 128, 513)
    p["xo"] = np.ascontiguousarray(xo)
    return p


# ---------------------------------------------------------------------------
# Device kernel
# ---------------------------------------------------------------------------

def build_nc(t_steps):
    from contextlib import ExitStack
    import concourse.bacc as bacc
    import concourse.mybir as mybir
    import concourse.tile as tile
    from concourse.masks import make_identity

    f32 = mybir.dt.float32
    f16 = mybir.dt.float16
    AF = mybir.ActivationFunctionType
    ALU = mybir.AluOpType

    t_total = t_steps
    nblk = t_total // BLK
    nc = bacc.Bacc("TRN2", target_bir_lowering=False)

    # ---- DRAM I/O -------------------------------------------------------
    Wh_d = nc.dram_tensor("Wh", [128, 3, 4, 4, 384], f16, kind="ExternalInput")
    Wi_d = nc.dram_tensor("Wi", [128, 2, 4, 4, 384], f16, kind="ExternalInput")
    Wx_d = nc.dram_tensor("Wx", [128, 5, 4, 384], f16, kind="ExternalInput")
    b23_d = nc.dram_tensor("b23", [1, 2, 4, 384], f16, kind="ExternalInput")
    bc_d = nc.dram_tensor("bc", [128, 3, 128], f32, kind="ExternalInput")
    xb_d = nc.dram_tensor("xb", [nblk, 128, 5, 128], f16, kind="ExternalInput")
    Wl_d = nc.dram_tensor("Wl", [128, 2, 4, 640], f16, kind="ExternalInput")
    bl_d = nc.dram_tensor("bl", [1, 2, 640], f16, kind="ExternalInput")
    xo_d = nc.dram_tensor("xo", [nblk, 128, 640], f32, kind="ExternalInput")
    o1_d = nc.dram_tensor("o1", [nblk, 128, 640], f32, kind="ExternalOutput")
    o2_d = nc.dram_tensor("o2", [nblk, 128, 640], f32, kind="ExternalOutput")

    with ExitStack() as ctx:
        tc = ctx.enter_context(tile.TileContext(nc))

        consts = ctx.enter_context(tc.tile_pool(name="consts", bufs=1))
        ident = consts.tile([128, 128], f32)
        make_identity(nc, ident)
        ones = consts.tile([1, 128], f16)
        nc.vector.memset(ones, 1.0)
        zeros = consts.tile([128, 128], f32)
        nc.vector.memset(zeros, 0.0)

        dram = ctx.enter_context(tc.tile_pool(name="dram", bufs=1, space="DRAM"))
        h3T = dram.tile([nblk, 128, BLK, 128], f16)

        with ExitStack() as rctx:
            wrec = rctx.enter_context(tc.tile_pool(name="wrec", bufs=1))
            Wh = wrec.tile([128, 3, 4, 4, 384], f16)
            nc.sync.dma_start(out=Wh, in_=Wh_d[:, :, :, :, :])
            Wi = wrec.tile([128, 2, 4, 4, 384], f16)
            nc.sync.dma_start(out=Wi, in_=Wi_d[:, :, :, :, :])
            Wx = wrec.tile([128, 5, 4, 384], f16)
            nc.sync.dma_start(out=Wx, in_=Wx_d[:, :, :, :])
            b23 = wrec.tile([1, 2, 4, 384], f16)
            nc.sync.dma_start(out=b23, in_=b23_d[:, :, :, :])
            bc = wrec.tile([128, 3, 128], f32)
            nc.sync.dma_start(out=bc, in_=bc_d[:, :, :])

            xpool = rctx.enter_context(tc.tile_pool(name="xpool", bufs=3))
            hTs_p = rctx.enter_context(tc.tile_pool(name="hTs", bufs=3))
            hLp = rctx.enter_context(tc.tile_pool(name="hLp", bufs=2))
            gpool = rctx.enter_context(tc.tile_pool(name="gpool", bufs=2))
            pap = rctx.enter_context(tc.tile_pool(name="pap", bufs=1, space="PSUM"))
            pbp = rctx.enter_context(tc.tile_pool(name="pbp", bufs=1, space="PSUM"))
            tpp = rctx.enter_context(tc.tile_pool(name="tpp", bufs=2, space="PSUM"))

            # per-layer state: transposed-h stacks keyed by (layer, block)
            hT_stacks = {}
            hL_cur = {1: None, 2: None, 3: None}   # [128,128] f32 last h
            xb_cur = {}

            def emit_step(l, blk, tp_idx, first, xbt):
                """One GRU step for layer l; tp_idx = t % BLK.

                PA [128,384] accumulates gi (+bias) on all 384 cols and the
                recurrent r|z part on cols 0:256.  PB [128,128] holds the
                recurrent n part (kept separate for the r* product).
                """
                li = l - 1
                hL_prev = hL_cur[l] if not first else zeros

                PA = pap.tile([128, 384], f32, tag=f"pa{l}")

                # ---- input side ----
                if l == 1:
                    for kt in range(5):
                        kk = 128 if kt < 4 else 2
                        for q in range(4):
                            nc.tensor.matmul(
                                PA[32 * q:32 * q + 32, :],
                                xbt[:kk, kt, 32 * tp_idx:32 * tp_idx + 32],
                                Wx[:kk, kt, q, :],
                                start=(kt == 0), stop=(first and kt == 4),
                                tile_position=(0, 32 * q),
                                skip_group_check=True)
                else:
                    hT_in = hT_stacks[(l - 1, blk)]
                    for q in range(4):
                        nc.tensor.matmul(
                            PA[32 * q:32 * q + 32, :],
                            ones[0:1, 32 * q:32 * q + 32],
                            b23[0:1, l - 2, q, :],
                            start=True, stop=False,
                            tile_position=(0, 32 * q),
                            skip_group_check=True)
                    for kt in range(4):
                        for q in range(4):
                            nc.tensor.matmul(
                                PA[32 * q:32 * q + 32, :],
                                hT_in[:, tp_idx, 32 * kt:32 * kt + 32],
                                Wi[:, l - 2, kt, q, :],
                                start=False, stop=(first and kt == 3),
                                tile_position=(0, 32 * q),
                                skip_group_check=True)

                # ---- recurrent side ----
                if not first:
                    PB = pbp.tile([128, 128], f32, tag=f"pb{l}")
                    if tp_idx > 0:
                        hT_own = hT_stacks[(l, blk)][:, tp_idx - 1, :]
                    else:
                        hT_own = hT_stacks[(l, blk - 1)][:, BLK - 1, :]
                    for kt in range(4):
                        for q in range(4):
                            nc.tensor.matmul(
                                PA[32 * q:32 * q + 32, 0:256],
                                hT_own[:, 32 * kt:32 * kt + 32],
                                Wh[:, li, kt, q, 0:256],
                                start=False, stop=(kt == 3),
                                tile_position=(0, 32 * q),
                                skip_group_check=True)
                            nc.tensor.matmul(
                                PB[32 * q:32 * q + 32, :],
                                hT_own[:, 32 * kt:32 * kt + 32],
                                Wh[:, li, kt, q, 256:384],
                                start=(kt == 0), stop=(kt == 3),
                                tile_position=(0, 32 * q))

                # ---- gates ----
                rz = gpool.tile([128, 256], f32, tag=f"rz{l}")
                nc.scalar.activation(rz, PA[:, 0:256], AF.Sigmoid)

                rn = gpool.tile([128, 128], f32, tag=f"rn{l}")
                if first:
                    nc.vector.tensor_mul(rn, rz[:, 0:128], bc[:, li, :])
                else:
                    p1n = gpool.tile([128, 128], f32, tag=f"p1n{l}")
                    nc.vector.tensor_add(p1n, PB, bc[:, li, :])
                    nc.vector.tensor_mul(rn, rz[:, 0:128], p1n)
                n = gpool.tile([128, 128], f32, tag=f"n{l}")
                nc.vector.tensor_add(n, rn, PA[:, 256:384])
                nc.scalar.activation(n, n, AF.Tanh)

                d = gpool.tile([128, 128], f32, tag=f"d{l}")
                nc.vector.tensor_sub(d, hL_prev, n)
                zd = gpool.tile([128, 128], f32, tag=f"zd{l}")
                nc.vector.tensor_mul(zd, rz[:, 128:256], d)
                hL = hLp.tile([128, 128], f32, tag=f"hL{l}")
                nc.vector.tensor_add(hL, n, zd)
                hL_cur[l] = hL

            def emit_transpose(l, blk, tp_idx):
                """Deferred: emitted after the other layers' matmul bursts so
                the PE queue doesn't stall waiting for this cell's gates."""
                tp = tpp.tile([128, 128], f32, tag="tp")
                nc.tensor.transpose(tp, hL_cur[l], ident)
                stack = hT_stacks[(l, blk)]
                if l == 2:
                    nc.vector.tensor_copy(stack[:, tp_idx, :], tp)
                else:
                    nc.scalar.copy(stack[:, tp_idx, :], tp)

            # ---- wavefront over supersteps ----
            def load_xb(blk):
                xbt = xpool.tile([128, 5, 128], f16, tag="xb")
                nc.sync.dma_start(out=xbt, in_=xb_d[blk, :, :, :])
                xb_cur[blk] = xbt
                xb_cur.pop(blk - 3, None)

            load_xb(0)
            for s in range(nblk + 2):
                if s + 1 < nblk:
                    load_xb(s + 1)
                active = [l for l in (1, 2, 3) if 0 <= s - (l - 1) < nblk]
                for l in active:
                    blk = s - (l - 1)
                    hT_stacks[(l, blk)] = hTs_p.tile(
                        [128, BLK, 128], f16, tag=f"hTs{l}", name=f"hTs{l}")
                    hT_stacks.pop((l, blk - 3), None)
                # interleave cells across layers so each layer's serial
                # gate-chain latency is covered by the other layers' matmuls;
                # transposes are emitted after the whole round of bursts
                for tp_idx in range(BLK):
                    for l in active:
                        blk = s - (l - 1)
                        t = BLK * blk + tp_idx
                        emit_step(l, blk, tp_idx, first=(t == 0),
                                  xbt=xb_cur.get(blk))
                    for l in active:
                        emit_transpose(l, s - (l - 1), tp_idx)
                if 3 in active:
                    blk = s - 2
                    nc.sync.dma_start(out=h3T[blk, :, :, :], in_=hT_stacks[(3, blk)])

        # ---- output phase ----------------------------------------------
        with ExitStack() as octx:
            wout = octx.enter_context(tc.tile_pool(name="wout", bufs=1))
            Wl = wout.tile([128, 2, 4, 640], f16)
            nc.sync.dma_start(out=Wl, in_=Wl_d[:, :, :, :])
            bl = wout.tile([1, 2, 640], f16)
            nc.sync.dma_start(out=bl, in_=bl_d[:, :, :])

            hpo = octx.enter_context(tc.tile_pool(name="hpo", bufs=3))
            xop = octx.enter_context(tc.tile_pool(name="xop", bufs=2))
            sp = octx.enter_context(tc.tile_pool(name="sp", bufs=2))
            osum = octx.enter_context(tc.tile_pool(name="osum", bufs=2, space="PSUM"))

            for blk in range(nblk):
                # relayout so chunk kt is contiguous: hT3[k, kt, t', b]
                hT3 = hpo.tile([128, 4, BLK, 32], f16, tag="hT3")
                for kt in range(4):
                    nc.sync.dma_start(
                        out=hT3[:, kt, :, :],
                        in_=h3T[blk, :, :, 32 * kt:32 * kt + 32])
                xo = xop.tile([128, 640], f32, tag="xo")
                nc.sync.dma_start(out=xo, in_=xo_d[blk, :, :])

                ss = []
                for hd in range(2):
                    psa = osum.tile([128, 512], f32, tag=f"psa{hd}")
                    psb = osum.tile([128, 128], f32, tag=f"psb{hd}")
                    nc.tensor.matmul(psa, ones[0:1, :], bl[0:1, hd, 0:512],
                                     start=True, stop=False)
                    nc.tensor.matmul(psb, ones[0:1, :], bl[0:1, hd, 512:640],
                                     start=True, stop=False)
                    for kt in range(4):
                        hs = hT3[:, kt, :, :].rearrange("k t b -> k (t b)")
                        nc.tensor.matmul(psa, hs, Wl[:, hd, kt, 0:512],
                                         start=False, stop=(kt == 3))
                        nc.tensor.matmul(psb, hs, Wl[:, hd, kt, 512:640],
                                         start=False, stop=(kt == 3))
                    s = sp.tile([128, 640], f32, tag=f"s{hd}")
                    nc.vector.tensor_scalar_max(s[:, 0:512], psa, 0.0)
                    nc.vector.tensor_scalar_max(s[:, 512:640], psb, 0.0)
                    ss.append(s)

                den = sp.tile([128, 640], f32, tag="den")
                nc.vector.tensor_add(den, ss[0], ss[1])
                nc.vector.tensor_scalar_add(den, den, 1e-16)
                lden = sp.tile([128, 640], f32, tag="lden")
                nc.scalar.activation(lden, den, AF.Ln)
                rden = sp.tile([128, 640], f32, tag="rden")
                nc.scalar.activation(rden, lden, AF.Exp, scale=-1.0)
                xr = sp.tile([128, 640], f32, tag="xr")
                nc.vector.tensor_mul(xr, xo, rden)
                for hd, od in ((0, o1_d), (1, o2_d)):
                    o = sp.tile([128, 640], f32, tag=f"o{hd}")
                    nc.vector.tensor_mul(o, ss[hd], xr)
                    nc.sync.dma_start(out=od[blk, :, :], in_=o)

    nc.finalize()
    return nc


# ---------------------------------------------------------------------------
# Runner (compiled-executable cache, SPMD via shard_map)
# ---------------------------------------------------------------------------

class _Runner:
    def __init__(self, nc, n_cores):
        import jax
        import concourse.mybir as mybir
        from concourse import bass2jax
        from concourse.bass2jax import (
            _bass_exec_p, install_neuronx_cc_hook, partition_id_tensor)
        from jax.experimental.shard_map import shard_map
        from jax.sharding import Mesh, PartitionSpec

        install_neuronx_cc_hook()
        self.jax = jax
        self.nc = nc
        self.n_cores = n_cores
        partition_name = (nc.partition_id_tensor.name
                          if nc.partition_id_tensor else None)
        in_names, out_names, out_avals, zero_outs = [], [], [], []
        for alloc in nc.m.functions[0].allocations:
            if not isinstance(alloc, mybir.MemoryLocationSet):
                continue
            name = alloc.memorylocations[0].name
            if alloc.kind == "ExternalInput":
                if name != partition_name:
                    in_names.append(name)
            elif alloc.kind == "ExternalOutput":
                shape = tuple(alloc.tensor_shape)
                dtype = mybir.dt.np(alloc.dtype)
                out_names.append(name)
                out_avals.append(jax.core.ShapedArray(shape, dtype))
                zero_outs.append(np.zeros(shape, dtype))
        n_params = len(in_names)
        self.in_names = list(in_names)
        self.out_names = out_names
        self.out_avals = out_avals
        self.zero_outs = zero_outs
        all_in = in_names + out_names
        if partition_name is not None:
            all_in.append(partition_name)

        def _body(*args):
            operands = list(args)
            if partition_name is not None:
                operands.append(partition_id_tensor())
            return tuple(_bass_exec_p.bind(
                *operands, out_avals=tuple(out_avals), in_names=tuple(all_in),
                out_names=tuple(out_names), lowering_input_output_aliases=(),
                sim_require_finite=True, sim_require_nnan=True, nc=nc))

        devices = jax.devices()[:n_cores]
        self.mesh = Mesh(np.asarray(devices), ("core",))
        self.pspec = PartitionSpec("core")
        n_out = len(out_names)
        self.sharded = jax.jit(
            shard_map(_body, mesh=self.mesh,
                      in_specs=(self.pspec,) * (n_params + n_out),
                      out_specs=(self.pspec,) * n_out,
                      check_rep=False),
            keep_unused=True)

    def prepare(self, in_map):
        import jax
        from jax.sharding import NamedSharding
        sh = NamedSharding(self.mesh, self.pspec)
        args = [np.concatenate([np.asarray(in_map[n])] * self.n_cores, axis=0)
                for n in self.in_names]
        args += [np.zeros((self.n_cores * z.shape[0], *z.shape[1:]), z.dtype)
                 for z in self.zero_outs]
        return [jax.device_put(a, sh) for a in args]

    def call(self, concat_in):
        return self.sharded(*concat_in)

    def results0(self, outs):
        """Core-0 slice of each output (sliced on device before transfer)."""
        res = {}
        for i, name in enumerate(self.out_names):
            n0 = self.out_avals[i].shape[0]
            res[name] = np.asarray(outs[i][:n0])
        return res


def _get_runner(t_steps):
    if t_steps not in _CACHE:
        _CACHE[t_steps] = _Runner(build_nc(t_steps), NCORES)
    return _CACHE[t_steps]


def _run(inputs, t_steps=T, time_reps=0):
    import time as _time
    r = _get_runner(t_steps)
    p = prep_inputs(inputs, t_steps)
    concat_in = r.prepare(p)
    outs = r.call(concat_in)  # first call compiles
    out = r.results0(outs)
    o1 = _unpack_out(out["o1"], t_steps)
    o2 = _unpack_out(out["o2"], t_steps)

    times = []
    for _ in range(time_reps):
        t0 = _time.time()
        outs = r.call(concat_in)
        for o in outs:
            o.block_until_ready()
        times.append(_time.time() - t0)
    return (o1, o2), times


def _unpack_out(o, t_steps):
    """[nblk, 32t'+b, 640] -> [B, T, 513]."""
    nblk = t_steps // BLK
    o = o.reshape(nblk, BLK, B, 640)[:, :, :, :F]
    return np.ascontiguousarray(np.transpose(o, (2, 0, 1, 3)).reshape(B, t_steps, F))


def kernel(**inputs):
    (o1, o2), _ = _run(inputs, T)
    return (o1, o2)
